# revision 30
# baseline (speedup 1.0000x reference)
"""GQA attention (16 q heads / 4 kv heads, HD=128, S=4096, D=2048) with RoPE,
causal mask, log-gate on kv positions, softmax, and output projection —
distributed over 8 NeuronCores.

Sharding: head-parallel. Core c computes q heads {2c, 2c+1} and kv head c//2.
Wq/Wk/Wv split column-wise, Wo row-wise; each core produces a partial [S, D]
output (fp16); host sums the 8 partials in fp32.

Single merged pipeline on-device (v4):
 - Attention computed transposed (scores^T [j, i], j = keys on partitions).
 - log(gate) - shift applied as the exp activation's per-partition bias, so
   denominators are plain sums of ex: accumulated on DVE in fp16 (2x mode),
   summed across partitions by M=1 matmuls, reciprocal via
   reciprocal_approx_fast, partition_broadcast on GpSimd.
 - Both heads share each scores PSUM tile ([128, 2, 256]) so one activation
   instruction computes exp for both heads of a key block (one bias column).
 - Diagonal key blocks restricted to their valid i-range; a single [128,128]
   triangular mask tile handles the diagonal itself. Upper-triangle blocks
   skipped entirely.
 - Per-half finalization: the out accumulation uses one PSUM bank per
   256-column half; each half's denominator/recip/broadcast/norm chain is
   emitted a slot after the half's last key block, so Wo output blocks become
   available mid-unit and the end-of-kernel tail only drains the final half.
 - Partial outputs stored/DMAed as fp16 (halves output HBM traffic).
 - Projections for chunk nb+1 and Wo/output evacuation are emitted as fillers
   inside the attention slot stream, keeping the PE continuously busy
   (p-state ramp to 2.4 GHz) and overlapping every engine.
"""

import math
from contextlib import ExitStack

import numpy as np

import concourse.bass as bass
import concourse.mybir as mybir
import concourse.tile as tile
from concourse import bacc, bass_isa
from concourse._compat import with_exitstack
from concourse.bass import ds
from concourse.bass_utils import run_bass_kernel_spmd
from concourse.masks import make_identity

P = 128
F = 512            # q-chunk per unit
HF = 256           # i-half per scores slot (1 PSUM bank for both heads)
S = 4096
D = 2048
HD = 128
KO = D // P        # 16 k-chunks for the projections
NB = S // F        # 8 sequence chunks
NJB = S // P       # 32 key blocks
F32 = mybir.dt.float32
BF16 = mybir.dt.bfloat16
FP16 = mybir.dt.float16
MULT = mybir.AluOpType.mult
ADD = mybir.AluOpType.add
EXPF = mybir.ActivationFunctionType.Exp


@with_exitstack
def _body(ctx: ExitStack, tc: tile.TileContext, io: dict):
    nc = tc.nc

    persist = ctx.enter_context(tc.tile_pool(name="persist", bufs=1))
    qT = persist.tile([P, 2, S], BF16, tag="qT")        # [d, h, i]
    kT = persist.tile([P, S], BF16, tag="kT")           # [d, j]
    vv = persist.tile([P, NJB, HD], FP16, tag="vv")     # [j, jb, d]
    attnT = persist.tile([P, 2, S], BF16, tag="attnT")  # [d, h, i] normalized
    loggate = persist.tile([P, NJB], F32, tag="lg")     # log(g)+shift [j, jb]
    tri = persist.tile([P, P], F32, tag="tri")          # 0 / -1e30 triangle
    ident = persist.tile([P, P], BF16, tag="ident")

    wpool = ctx.enter_context(tc.tile_pool(name="wpool", bufs=1))
    wq = wpool.tile([P, KO, 2 * HD], BF16, tag="wq")
    wq_r = io["wq"].rearrange("(ko p) m -> p ko m", p=P)
    wk = wpool.tile([P, KO, HD], BF16, tag="wk")
    wk_r = io["wk"].rearrange("(ko p) m -> p ko m", p=P)
    wv = wpool.tile([P, KO, HD], BF16, tag="wv")
    wv_r = io["wv"].rearrange("(ko p) m -> p ko m", p=P)
    wo = wpool.tile([P, 2, D], BF16, tag="wo")
    ones = persist.tile([P, 1], FP16, tag="ones")
    make_identity(nc, ident[:])
    nc.vector.memset(ones[:], 1.0)

    xt_r = io["xt"].rearrange("(ko p) s -> p ko s", p=P)  # [128, 16, 4096]

    xt_pool = ctx.enter_context(tc.tile_pool(name="xt", bufs=12))
    tab_pool = ctx.enter_context(tc.tile_pool(name="tab", bufs=4))
    rope_pool = ctx.enter_context(tc.tile_pool(name="rope", bufs=2))
    vt_pool = ctx.enter_context(tc.tile_pool(name="vt", bufs=2))
    ex_pool = ctx.enter_context(tc.tile_pool(name="ex", bufs=6))
    acc_pool = ctx.enter_context(tc.tile_pool(name="acc", bufs=2))
    r2_pool = ctx.enter_context(tc.tile_pool(name="r2", bufs=2))
    rbc_pool = ctx.enter_context(tc.tile_pool(name="rbc", bufs=2))
    ob_pool = ctx.enter_context(tc.tile_pool(name="ob", bufs=6))
    psSc = ctx.enter_context(tc.tile_pool(name="psSc", bufs=2, space="PSUM"))
    psOut = ctx.enter_context(tc.tile_pool(name="psOut", bufs=1, space="PSUM"))
    psProj = ctx.enter_context(tc.tile_pool(name="psProj", bufs=1, space="PSUM"))
    psT = ctx.enter_context(tc.tile_pool(name="psT", bufs=1, space="PSUM"))
    psWo = ctx.enter_context(tc.tile_pool(name="psWo", bufs=2, space="PSUM"))

    # ------- projection machinery: per-chunk work as a list of emission
    # closures (filler items for the attention slot stream) -------
    def make_proj_items(c, fine=False):
        """Emission closures computing qT/kT/vv for sequence chunk c."""
        sl = ds(c * F, F)
        state = {}

        def dma_x():
            sub = 1 if fine else 2
            xq = []
            for xi in range(4):
                xtile = xt_pool.tile([P, 4, F], BF16, tag="xt")
                for h in range(4 // sub):  # split across DMA queues
                    nc.sync.dma_start(
                        xtile[:, ds(h * sub, sub), :],
                        xt_r[:, ds(xi * 4 + h * sub, sub), sl],
                    )
                xq.append(xtile)
            tabs = tab_pool.tile([P, 4, F], BF16, tag="tabs")
            for h in range(4 // sub):
                nc.sync.dma_start(
                    tabs[:, ds(h * sub, sub), :], io["tabs"][:, ds(h * sub, sub), sl]
                )
            state["xq"] = xq
            state["tabs"] = tabs

        def mm_group(w_sb, m0, g, key):
            def emit():
                if g == 0:
                    state[key] = psProj.tile([P, F], F32, tag="ps", name="ps")
                ps = state[key]
                for ko in range(g * 4, g * 4 + 4):
                    nc.tensor.matmul(
                        ps[:],
                        lhsT=w_sb[:, ko, ds(m0, P)],
                        rhs=state["xq"][ko // 4][:, ko % 4, :],
                        start=(ko == 0),
                        stop=(ko == KO - 1),
                    )
            return emit

        def rope_fin(key, trow, dest):
            def emit():
                ps = state[key]
                tabs = state["tabs"]
                tmp = rope_pool.tile([P, F], BF16, tag="tmp")
                nc.scalar.copy(tmp[:], ps[:])
                rot = rope_pool.tile([P, F], BF16, tag="rot")
                nc.sync.dma_start(rot[0:64, :], tmp[64:128, :])
                nc.sync.dma_start(rot[64:128, :], tmp[0:64, :])
                t1 = rope_pool.tile([P, F], BF16, tag="t1")
                nc.vector.tensor_tensor(t1[:], tmp[:], tabs[:, trow, :], op=MULT)
                r2 = rope_pool.tile([P, F], BF16, tag="r2t")
                nc.vector.tensor_tensor(r2[:], rot[:], tabs[:, trow + 1, :], op=MULT)
                nc.vector.tensor_tensor(dest, t1[:], r2[:], op=ADD)
            return emit

        def v_fin():
            ps = state["v"]
            vT = vt_pool.tile([P, F], BF16, tag="vT")
            nc.scalar.copy(vT[:], ps[:])
            for isub in range(4):
                pt = psT.tile([P, P], BF16, tag="pt")
                nc.tensor.transpose(pt[:], vT[:, ds(isub * P, P)], ident[:])
                nc.scalar.copy(vv[:, c * 4 + isub, :], pt[:])

        items = [dma_x]
        for g in range(4):
            items.append(mm_group(wq, 0, g, "q0"))
        items.append(rope_fin("q0", 0, qT[:, 0, sl]))
        for g in range(4):
            items.append(mm_group(wq, P, g, "q1"))
        items.append(rope_fin("q1", 0, qT[:, 1, sl]))
        for g in range(4):
            items.append(mm_group(wk, 0, g, "k"))
        items.append(rope_fin("k", 2, kT[:, sl]))
        for g in range(4):
            items.append(mm_group(wv, 0, g, "v"))
        items.append(v_fin)
        return items

    # ------- Wo output machinery -------
    wo_q = []  # (nb, i2, e) output blocks, appended once their norm is emitted
    wo_alt = [0]
    po_fine = [False]  # final unit: split output DMAs finer for a short tail

    def emit_po():
        _, i2, e = wo_q.pop(0)
        po = psWo.tile([P, F], F32, tag="po")
        for hh in range(2):
            nc.tensor.matmul(
                po[:],
                lhsT=attnT[:, hh, ds(i2 * P, P)],
                rhs=wo[:, hh, ds(e * F, F)],
                start=(hh == 0),
                stop=(hh == 1),
            )
        ob = ob_pool.tile([P, F], FP16, tag="ob")
        if wo_alt[0] % 2 == 0:
            nc.scalar.copy(ob[:], po[:])
        else:
            nc.vector.tensor_copy(ob[:], po[:])
        wo_alt[0] += 1
        nsp = 4 if po_fine[0] else 2
        for h2 in range(nsp):
            nc.sync.dma_start(
                io["outp"][ds(i2 * P, P), ds(e * F + h2 * (F // nsp), F // nsp)],
                ob[:, ds(h2 * (F // nsp), F // nsp)],
            )

    # ------- prologue -------
    # DMA issue order matters: the first projection group (q0 g0/g1) needs
    # wq ko0-7 m0-255 and x chunk-0 ko0-7; put those on the first queues so
    # they land together in the first DMA round.
    proj_items = make_proj_items(0, fine=True)
    dma_x0 = proj_items.pop(0)
    for g in range(4):
        nc.sync.dma_start(wq[:, ds(g * 2, 2), :], wq_r[:, ds(g * 2, 2), :])
    for h in range(4):
        nc.sync.dma_start(wk[:, ds(h * 4, 4), :], wk_r[:, ds(h * 4, 4), :])
    dma_x0()  # 16 x sub-DMAs + 4 tabs
    for g in range(4, 8):
        nc.sync.dma_start(wq[:, ds(g * 2, 2), :], wq_r[:, ds(g * 2, 2), :])
    for h in range(4):
        nc.sync.dma_start(wv[:, ds(h * 4, 4), :], wv_r[:, ds(h * 4, 4), :])
    nc.sync.dma_start(loggate[:], io["loggate"])
    nc.sync.dma_start(tri[:], io["tri"])
    # chunk-0 emission order: q0 then k first (k's weights are small and land
    # in the first DMA round, covering the second x-DMA round's latency)
    items0 = proj_items
    reordered = items0[0:5] + items0[10:15] + items0[5:10] + items0[15:]
    for it in reordered:
        it()  # chunk 0 projections up front
    wo_rr = io["wo"].rearrange("(h p) e -> p h e", p=P)
    for g in range(4):
        nc.sync.dma_start(wo[:, :, ds(g * F, F)], wo_rr[:, :, ds(g * F, F)])
    proj_items = make_proj_items(1)
    proj_items.pop(0)()  # chunk-1 input DMA issued ahead of unit 0
    carry = []  # (delay_slots, closure) finalization steps from previous unit

    for nb in range(NB):
        out2 = [
            psOut.tile([P, 2, HF], F32, tag=f"oi{ih}", name=f"oi{ih}")
            for ih in range(2)
        ]
        ACC = acc_pool.tile([P, 2, F], FP16, tag="acc")
        pend = []  # out matmuls trail scores by 2 slots
        jmax0 = 4 * nb + 2
        nslots = jmax0 + 4 * nb + 4
        sched = {}
        for d, fn in carry:
            sched.setdefault(d, []).append(fn)
        carry = []

        def emit_out(jb, ex, ih, i0, w, last):
            for hh in range(2):
                nc.tensor.matmul(
                    out2[ih][:, hh, ds(i0, w)],
                    lhsT=vv[:, jb, :],
                    rhs=ex[:, hh, ds(i0, w)],
                    start=(jb == 0 and hh == 0),
                    stop=(last and hh == 1),
                )

        # per-half finalization chain: M=1 matmuls sum ACC over partitions
        # (both heads into one PSUM row), DVE reciprocal, one GpSimd
        # broadcast, then norm into attnT.
        def make_chain(ih, nb=nb, ACC=ACC, out2=out2):
            st = {}

            def den_step():
                den = psWo.tile([P, F], F32, tag="po", name=f"den{ih}")
                for hh in range(2):
                    nc.tensor.matmul(
                        den[0:1, ds(hh * HF, HF)],
                        lhsT=ones[:],
                        rhs=ACC[:, hh, ds(ih * HF, HF)],
                        start=(hh == 0),
                        stop=(hh == 1),
                    )
                r2 = r2_pool.tile([1, F], F32, tag="r2", name=f"r2_{ih}")
                nc.vector.reciprocal_approx_fast(r2[:], den[0:1, :])
                rbc = rbc_pool.tile([P, 2, HF], F32, tag="rbc")
                nc.gpsimd.partition_broadcast(rbc[:], r2[:])
                st["rbc"] = rbc

            def norm_step():
                nc.vector.tensor_tensor(
                    attnT[:, :, ds(nb * F + ih * HF, HF)],
                    out2[ih][:],
                    st["rbc"][:],
                    op=MULT,
                )

            def wo_step():
                base = nb * 4 + ih * 2
                wo_q.extend((nb, base + k, e) for k in range(2) for e in range(4))

            return den_step, norm_step, wo_step

        den0, norm0, wo0 = make_chain(0)
        sched.setdefault(jmax0 + 1, []).append(den0)
        sched.setdefault(jmax0 + 4, []).append(norm0)
        sched.setdefault(jmax0 + 5, []).append(wo0)

        slot = 0
        fill_alt = [0]

        def filler():
            # alternate between Wo output blocks and projection items
            a, b = (0, 1) if fill_alt[0] % 2 == 0 else (1, 0)
            for which in (a, b):
                if which == 0 and wo_q:
                    emit_po()
                    fill_alt[0] += 1
                    return True
                if which == 1 and proj_items:
                    proj_items.pop(0)()
                    fill_alt[0] += 1
                    return True
            return False

        for ih in range(2):
            jmax = 4 * nb + 2 * ih + 2
            for jb in range(jmax):
                for fn in sched.pop(slot, ()):
                    fn()
                dp = jb - (4 * nb + 2 * ih)
                i0 = max(0, dp) * P
                w = HF - i0
                qoff = nb * F + ih * HF + i0
                # scores for both heads -> one PSUM bank
                sc = psSc.tile([P, 2, HF], F32, tag="sc")
                for hh in range(2):
                    nc.tensor.matmul(
                        sc[:, hh, ds(i0, w)],
                        lhsT=kT[:, ds(jb * P, P)],
                        rhs=qT[:, hh, ds(qoff, w)],
                        start=(hh == 0),
                        stop=(hh == 1),
                    )
                if dp >= 0:  # diagonal block: triangular mask
                    for hh in range(2):
                        nc.vector.tensor_tensor(
                            sc[:, hh, ds(i0, P)], sc[:, hh, ds(i0, P)], tri[:],
                            op=ADD,
                        )
                ex = ex_pool.tile([P, 2, HF], FP16, tag="ex")
                nc.scalar.activation(
                    ex[:, :, ds(i0, w)], sc[:, :, ds(i0, w)], EXPF,
                    bias=loggate[:, jb : jb + 1],
                )
                while len(pend) >= (3 if slot < 4 else 2):
                    emit_out(*pend.pop(0))
                if jb == 0:
                    nc.vector.tensor_copy(ACC[:, :, ds(ih * HF, HF)], ex[:])
                else:
                    nc.vector.tensor_tensor(
                        ACC[:, :, ds(ih * HF + i0, w)],
                        ACC[:, :, ds(ih * HF + i0, w)],
                        ex[:, :, ds(i0, w)],
                        op=ADD,
                    )
                pend.append((jb, ex, ih, i0, w, jb == jmax - 1))
                # fillers: scale count to remaining backlog
                backlog = len(wo_q) + len(proj_items)
                remaining = max(1, nslots - slot)
                nfill = 1 if backlog <= remaining else 2
                for _ in range(nfill):
                    if not filler():
                        break
                slot += 1

        while pend:
            emit_out(*pend.pop(0))
        for s in sorted(sched):  # steps not reached inside the slot loop
            for fn in sched[s]:
                fn()

        den1, norm1, wo1 = make_chain(1)
        for _ in range(2):  # filler cover for the ACC->den dependency
            if not filler():
                break
        den1()
        if nb == NB - 1:
            norm1()
            wo1()
            po_fine[0] = True
            while wo_q:
                emit_po()
        else:
            carry = [(0, norm1), (1, wo1)]
            # drain remaining projection items so chunk nb+1 is ready
            while proj_items:
                proj_items.pop(0)()
            if nb + 2 < NB:
                proj_items = make_proj_items(nb + 2)
                proj_items.pop(0)()  # chunk nb+2 input DMA a full unit early
            else:
                proj_items = []


_NC_CACHE = None


def build_nc():
    global _NC_CACHE
    if _NC_CACHE is not None:
        return _NC_CACHE
    nc = bacc.Bacc("TRN2", target_bir_lowering=False, debug=False)
    io = {
        "xt": nc.dram_tensor("xt", [D, S], BF16, kind="ExternalInput").ap(),
        "wq": nc.dram_tensor("wq", [D, 2 * HD], BF16, kind="ExternalInput").ap(),
        "wk": nc.dram_tensor("wk", [D, HD], BF16, kind="ExternalInput").ap(),
        "wv": nc.dram_tensor("wv", [D, HD], BF16, kind="ExternalInput").ap(),
        "wo": nc.dram_tensor("wo", [2 * HD, D], BF16, kind="ExternalInput").ap(),
        "tabs": nc.dram_tensor("tabs", [P, 4, S], BF16, kind="ExternalInput").ap(),
        "loggate": nc.dram_tensor("loggate", [P, NJB], F32, kind="ExternalInput").ap(),
        "tri": nc.dram_tensor("tri", [P, P], F32, kind="ExternalInput").ap(),
        "outp": nc.dram_tensor("outp", [S, D], FP16, kind="ExternalOutput").ap(),
    }
    with tile.TileContext(nc) as tc:
        _body(tc, io)
    nc.compile()
    _NC_CACHE = nc
    return nc


def make_in_maps(hidden_states, attention_mask, cos, sin, gate, Wq, Wk, Wv, Wo):
    import ml_dtypes
    bf16 = ml_dtypes.bfloat16
    X = np.asarray(hidden_states, np.float32).reshape(S, D)
    xt = np.ascontiguousarray(X.T.astype(bf16))
    cosT = np.ascontiguousarray(np.asarray(cos, np.float32).reshape(S, HD).T)
    sinT = np.ascontiguousarray(np.asarray(sin, np.float32).reshape(S, HD).T)
    sinTs = np.concatenate([-sinT[: HD // 2], sinT[HD // 2 :]], axis=0)
    sc = np.float32(1.0 / math.sqrt(HD))
    tabs = np.ascontiguousarray(
        np.stack([cosT * sc, sinTs * sc, cosT, sinTs], axis=1).astype(bf16)
    )
    # log gate with 2^-5 shift: keeps exp sums within fp16 range
    g = np.asarray(gate, np.float32).reshape(S) + np.float32(1e-8)
    lg = np.log(g).astype(np.float32) - np.float32(5.0 * math.log(2.0))
    loggate = np.ascontiguousarray(lg.reshape(NJB, P).T)
    jj = np.arange(P)[:, None]
    ii = np.arange(P)[None, :]
    tri = np.where(jj <= ii, np.float32(0), np.float32(-1e30))
    tri = np.ascontiguousarray(tri.astype(np.float32))

    Wq = np.asarray(Wq, np.float32)
    Wk = np.asarray(Wk, np.float32)
    Wv = np.asarray(Wv, np.float32)
    Wo = np.asarray(Wo, np.float32)

    in_maps = []
    for c in range(8):
        g128 = c // 2
        in_maps.append(
            {
                "xt": xt,
                "wq": np.ascontiguousarray(Wq[:, c * 256 : (c + 1) * 256].astype(bf16)),
                "wk": np.ascontiguousarray(Wk[:, g128 * HD : (g128 + 1) * HD].astype(bf16)),
                "wv": np.ascontiguousarray(Wv[:, g128 * HD : (g128 + 1) * HD].astype(bf16)),
                "wo": np.ascontiguousarray(Wo[c * 256 : (c + 1) * 256, :].astype(bf16)),
                "tabs": tabs,
                "loggate": loggate,
                "tri": tri,
            }
        )
    return in_maps


def kernel(hidden_states, attention_mask, cos, sin, gate, Wq, Wk, Wv, Wo,
           **kwargs):
    nc = build_nc()
    in_maps = make_in_maps(
        hidden_states, attention_mask, cos, sin, gate, Wq, Wk, Wv, Wo
    )
    res = run_bass_kernel_spmd(nc, in_maps, core_ids=list(range(8)), **kwargs)
    acc = res.results[0]["outp"].astype(np.float32)
    for c in range(1, 8):
        acc += res.results[c]["outp"].astype(np.float32)
    out = acc.reshape(1, S, D)
    if kwargs:
        return out, res
    return out


# revision 32
# speedup vs baseline: 1.1430x; 1.1430x over previous
"""GQA attention (16 q heads / 4 kv heads, HD=128, S=4096, D=2048) with RoPE,
causal mask, log-gate on kv positions, softmax, and output projection —
distributed over 8 NeuronCores.

Sharding: head-parallel. Core c computes q heads {2c, 2c+1} and kv head c//2.
Wq/Wk/Wv split column-wise, Wo row-wise; each core produces a partial [S, D]
output (fp16); host sums the 8 partials in fp32.

Single merged pipeline on-device (v4):
 - Attention computed transposed (scores^T [j, i], j = keys on partitions).
 - log(gate) - shift applied as the exp activation's per-partition bias, so
   denominators are plain sums of ex: accumulated on DVE in fp16 (2x mode),
   summed across partitions by M=1 matmuls, reciprocal via
   reciprocal_approx_fast, partition_broadcast on GpSimd.
 - Both heads share each scores PSUM tile ([128, 2, 256]) so one activation
   instruction computes exp for both heads of a key block (one bias column).
 - Diagonal key blocks restricted to their valid i-range; a single [128,128]
   triangular mask tile handles the diagonal itself. Upper-triangle blocks
   skipped entirely.
 - Per-half finalization: the out accumulation uses one PSUM bank per
   256-column half; each half's denominator/recip/broadcast/norm chain is
   emitted a slot after the half's last key block, so Wo output blocks become
   available mid-unit and the end-of-kernel tail only drains the final half.
 - Partial outputs stored/DMAed as fp16 (halves output HBM traffic).
 - Projections for chunk nb+1 and Wo/output evacuation are emitted as fillers
   inside the attention slot stream, keeping the PE continuously busy
   (p-state ramp to 2.4 GHz) and overlapping every engine.
"""

import math
from contextlib import ExitStack

import numpy as np

import concourse.bass as bass
import concourse.mybir as mybir
import concourse.tile as tile
from concourse import bacc, bass_isa
from concourse._compat import with_exitstack
from concourse.bass import ds
from concourse.bass_utils import run_bass_kernel_spmd
from concourse.masks import make_identity

P = 128
F = 512            # q-chunk per unit
HF = 256           # i-half per scores slot (1 PSUM bank for both heads)
S = 4096
D = 2048
HD = 128
KO = D // P        # 16 k-chunks for the projections
NB = S // F        # 8 sequence chunks
NJB = S // P       # 32 key blocks
F32 = mybir.dt.float32
BF16 = mybir.dt.bfloat16
FP16 = mybir.dt.float16
MULT = mybir.AluOpType.mult
ADD = mybir.AluOpType.add
EXPF = mybir.ActivationFunctionType.Exp


@with_exitstack
def _body(ctx: ExitStack, tc: tile.TileContext, io: dict):
    nc = tc.nc

    persist = ctx.enter_context(tc.tile_pool(name="persist", bufs=1))
    qT = persist.tile([P, 2, S], BF16, tag="qT")        # [d, h, i]
    kT = persist.tile([P, S], BF16, tag="kT")           # [d, j]
    vv = persist.tile([P, NJB, HD], FP16, tag="vv")     # [j, jb, d]
    attnT = persist.tile([P, 2, S], BF16, tag="attnT")  # [d, h, i] normalized
    loggate = persist.tile([P, NJB], F32, tag="lg")     # log(g)+shift [j, jb]
    tri = persist.tile([P, P], F32, tag="tri")          # 0 / -1e30 triangle
    ident = persist.tile([P, P], BF16, tag="ident")

    wpool = ctx.enter_context(tc.tile_pool(name="wpool", bufs=1))
    wq = wpool.tile([P, KO, 2 * HD], BF16, tag="wq")
    wq_r = io["wq"].rearrange("(ko p) m -> p ko m", p=P)
    wk = wpool.tile([P, KO, HD], BF16, tag="wk")
    wk_r = io["wk"].rearrange("(ko p) m -> p ko m", p=P)
    wv = wpool.tile([P, KO, HD], BF16, tag="wv")
    wv_r = io["wv"].rearrange("(ko p) m -> p ko m", p=P)
    wo = wpool.tile([P, 2, D], BF16, tag="wo")
    ones = persist.tile([P, 1], FP16, tag="ones")
    make_identity(nc, ident[:])
    nc.vector.memset(ones[:], 1.0)

    xt_r = io["xt"].rearrange("(ko p) s -> p ko s", p=P)  # [128, 16, 4096]

    xt_pool = ctx.enter_context(tc.tile_pool(name="xt", bufs=12))
    tab_pool = ctx.enter_context(tc.tile_pool(name="tab", bufs=4))
    rope_pool = ctx.enter_context(tc.tile_pool(name="rope", bufs=2))
    vt_pool = ctx.enter_context(tc.tile_pool(name="vt", bufs=2))
    ex_pool = ctx.enter_context(tc.tile_pool(name="ex", bufs=6))
    acc_pool = ctx.enter_context(tc.tile_pool(name="acc", bufs=2))
    r2_pool = ctx.enter_context(tc.tile_pool(name="r2", bufs=2))
    rbc_pool = ctx.enter_context(tc.tile_pool(name="rbc", bufs=2))
    ob_pool = ctx.enter_context(tc.tile_pool(name="ob", bufs=6))
    psSc = ctx.enter_context(tc.tile_pool(name="psSc", bufs=2, space="PSUM"))
    psOut = ctx.enter_context(tc.tile_pool(name="psOut", bufs=1, space="PSUM"))
    psProj = ctx.enter_context(tc.tile_pool(name="psProj", bufs=1, space="PSUM"))
    psT = ctx.enter_context(tc.tile_pool(name="psT", bufs=1, space="PSUM"))
    psWo = ctx.enter_context(tc.tile_pool(name="psWo", bufs=2, space="PSUM"))

    # ------- projection machinery: per-chunk work as a list of emission
    # closures (filler items for the attention slot stream) -------
    def make_proj_items(c, fine=False):
        """Emission closures computing qT/kT/vv for sequence chunk c."""
        sl = ds(c * F, F)
        state = {}

        def dma_x():
            sub = 1 if fine else 2
            xq = []
            for xi in range(4):
                xtile = xt_pool.tile([P, 4, F], BF16, tag="xt")
                for h in range(4 // sub):  # split across DMA queues
                    nc.sync.dma_start(
                        xtile[:, ds(h * sub, sub), :],
                        xt_r[:, ds(xi * 4 + h * sub, sub), sl],
                    )
                xq.append(xtile)
            tabs = tab_pool.tile([P, 4, F], BF16, tag="tabs")
            for h in range(4 // sub):
                nc.sync.dma_start(
                    tabs[:, ds(h * sub, sub), :], io["tabs"][:, ds(h * sub, sub), sl]
                )
            state["xq"] = xq
            state["tabs"] = tabs

        def mm_group(w_sb, m0, g, key):
            def emit():
                if g == 0:
                    state[key] = psProj.tile([P, F], F32, tag="ps", name="ps")
                ps = state[key]
                for ko in range(g * 4, g * 4 + 4):
                    nc.tensor.matmul(
                        ps[:],
                        lhsT=w_sb[:, ko, ds(m0, P)],
                        rhs=state["xq"][ko // 4][:, ko % 4, :],
                        start=(ko == 0),
                        stop=(ko == KO - 1),
                    )
            return emit

        def rope_fin(key, trow, dest):
            def emit():
                ps = state[key]
                tabs = state["tabs"]
                tmp = rope_pool.tile([P, F], BF16, tag="tmp")
                nc.scalar.copy(tmp[:], ps[:])
                rot = rope_pool.tile([P, F], BF16, tag="rot")
                nc.sync.dma_start(rot[0:64, :], tmp[64:128, :])
                nc.sync.dma_start(rot[64:128, :], tmp[0:64, :])
                t1 = rope_pool.tile([P, F], BF16, tag="t1")
                nc.vector.tensor_tensor(t1[:], tmp[:], tabs[:, trow, :], op=MULT)
                r2 = rope_pool.tile([P, F], BF16, tag="r2t")
                nc.vector.tensor_tensor(r2[:], rot[:], tabs[:, trow + 1, :], op=MULT)
                nc.vector.tensor_tensor(dest, t1[:], r2[:], op=ADD)
            return emit

        def v_fin():
            ps = state["v"]
            vT = vt_pool.tile([P, F], BF16, tag="vT")
            nc.scalar.copy(vT[:], ps[:])
            for isub in range(4):
                pt = psT.tile([P, P], BF16, tag="pt")
                nc.tensor.transpose(pt[:], vT[:, ds(isub * P, P)], ident[:])
                nc.scalar.copy(vv[:, c * 4 + isub, :], pt[:])

        items = [dma_x]
        for g in range(4):
            items.append(mm_group(wq, 0, g, "q0"))
        items.append(rope_fin("q0", 0, qT[:, 0, sl]))
        for g in range(4):
            items.append(mm_group(wq, P, g, "q1"))
        items.append(rope_fin("q1", 0, qT[:, 1, sl]))
        for g in range(4):
            items.append(mm_group(wk, 0, g, "k"))
        items.append(rope_fin("k", 2, kT[:, sl]))
        for g in range(4):
            items.append(mm_group(wv, 0, g, "v"))
        items.append(v_fin)
        return items

    # ------- Wo output machinery -------
    wo_q = []  # (nb, i2, e) output blocks, appended once their norm is emitted
    wo_alt = [0]
    po_fine = [False]  # final unit: split output DMAs finer for a short tail

    def emit_po():
        _, i2, e = wo_q.pop(0)
        po = psWo.tile([P, F], F32, tag="po")
        for hh in range(2):
            nc.tensor.matmul(
                po[:],
                lhsT=attnT[:, hh, ds(i2 * P, P)],
                rhs=wo[:, hh, ds(e * F, F)],
                start=(hh == 0),
                stop=(hh == 1),
            )
        ob = ob_pool.tile([P, F], FP16, tag="ob")
        if wo_alt[0] % 2 == 0:
            nc.scalar.copy(ob[:], po[:])
        else:
            nc.vector.tensor_copy(ob[:], po[:])
        wo_alt[0] += 1
        nsp = 4 if po_fine[0] else 2
        for h2 in range(nsp):
            nc.sync.dma_start(
                io["outp"][ds(i2 * P, P), ds(e * F + h2 * (F // nsp), F // nsp)],
                ob[:, ds(h2 * (F // nsp), F // nsp)],
            )

    # ------- prologue -------
    # DMA issue order matters: the first projection group (q0 g0/g1) needs
    # wq ko0-7 m0-255 and x chunk-0 ko0-7; put those on the first queues so
    # they land together in the first DMA round.
    proj_items = make_proj_items(0, fine=True)
    dma_x0 = proj_items.pop(0)
    for g in range(4):
        nc.sync.dma_start(wq[:, ds(g * 2, 2), :], wq_r[:, ds(g * 2, 2), :])
    for h in range(4):
        nc.sync.dma_start(wk[:, ds(h * 4, 4), :], wk_r[:, ds(h * 4, 4), :])
    dma_x0()  # 16 x sub-DMAs + 4 tabs
    for g in range(4, 8):
        nc.sync.dma_start(wq[:, ds(g * 2, 2), :], wq_r[:, ds(g * 2, 2), :])
    for h in range(4):
        nc.sync.dma_start(wv[:, ds(h * 4, 4), :], wv_r[:, ds(h * 4, 4), :])
    nc.sync.dma_start(loggate[:], io["loggate"])
    nc.sync.dma_start(tri[:], io["tri"])
    for it in proj_items:
        it()  # chunk 0 projections up front
    wo_rr = io["wo"].rearrange("(h p) e -> p h e", p=P)
    for g in range(4):
        nc.sync.dma_start(wo[:, :, ds(g * F, F)], wo_rr[:, :, ds(g * F, F)])
    proj_items = make_proj_items(1)
    proj_items.pop(0)()  # chunk-1 input DMA issued ahead of unit 0
    carry = []  # (delay_slots, closure) finalization steps from previous unit

    for nb in range(NB):
        out2 = [
            psOut.tile([P, 2, HF], F32, tag=f"oi{ih}", name=f"oi{ih}")
            for ih in range(2)
        ]
        ACC = acc_pool.tile([P, 2, F], FP16, tag="acc")
        pend = []  # out matmuls trail scores by 2 slots
        jmax0 = 4 * nb + 2
        nslots = jmax0 + 4 * nb + 4
        sched = {}
        for d, fn in carry:
            sched.setdefault(d, []).append(fn)
        carry = []

        def emit_out(jb, ex, ih, i0, w, last):
            for hh in range(2):
                nc.tensor.matmul(
                    out2[ih][:, hh, ds(i0, w)],
                    lhsT=vv[:, jb, :],
                    rhs=ex[:, hh, ds(i0, w)],
                    start=(jb == 0 and hh == 0),
                    stop=(last and hh == 1),
                )

        # per-half finalization chain: M=1 matmuls sum ACC over partitions
        # (both heads into one PSUM row), DVE reciprocal, one GpSimd
        # broadcast, then norm into attnT.
        def make_chain(ih, nb=nb, ACC=ACC, out2=out2):
            st = {}

            def den_step():
                den = psWo.tile([P, F], F32, tag="po", name=f"den{ih}")
                for hh in range(2):
                    nc.tensor.matmul(
                        den[0:1, ds(hh * HF, HF)],
                        lhsT=ones[:],
                        rhs=ACC[:, hh, ds(ih * HF, HF)],
                        start=(hh == 0),
                        stop=(hh == 1),
                    )
                r2 = r2_pool.tile([1, F], F32, tag="r2", name=f"r2_{ih}")
                nc.vector.reciprocal_approx_fast(r2[:], den[0:1, :])
                rbc = rbc_pool.tile([P, 2, HF], F32, tag="rbc")
                nc.gpsimd.partition_broadcast(rbc[:], r2[:])
                st["rbc"] = rbc

            def norm_step():
                nc.vector.tensor_tensor(
                    attnT[:, :, ds(nb * F + ih * HF, HF)],
                    out2[ih][:],
                    st["rbc"][:],
                    op=MULT,
                )

            def wo_step():
                base = nb * 4 + ih * 2
                wo_q.extend((nb, base + k, e) for k in range(2) for e in range(4))

            return den_step, norm_step, wo_step

        den0, norm0, wo0 = make_chain(0)
        sched.setdefault(jmax0 + 1, []).append(den0)
        sched.setdefault(jmax0 + 4, []).append(norm0)
        sched.setdefault(jmax0 + 5, []).append(wo0)

        slot = 0
        fill_alt = [0]

        def filler():
            # alternate between Wo output blocks and projection items
            a, b = (0, 1) if fill_alt[0] % 2 == 0 else (1, 0)
            for which in (a, b):
                if which == 0 and wo_q:
                    emit_po()
                    fill_alt[0] += 1
                    return True
                if which == 1 and proj_items:
                    proj_items.pop(0)()
                    fill_alt[0] += 1
                    return True
            return False

        for ih in range(2):
            jmax = 4 * nb + 2 * ih + 2
            for jb in range(jmax):
                for fn in sched.pop(slot, ()):
                    fn()
                dp = jb - (4 * nb + 2 * ih)
                i0 = max(0, dp) * P
                w = HF - i0
                qoff = nb * F + ih * HF + i0
                # scores for both heads -> one PSUM bank
                sc = psSc.tile([P, 2, HF], F32, tag="sc")
                for hh in range(2):
                    nc.tensor.matmul(
                        sc[:, hh, ds(i0, w)],
                        lhsT=kT[:, ds(jb * P, P)],
                        rhs=qT[:, hh, ds(qoff, w)],
                        start=(hh == 0),
                        stop=(hh == 1),
                    )
                if dp >= 0:  # diagonal block: triangular mask
                    for hh in range(2):
                        nc.vector.tensor_tensor(
                            sc[:, hh, ds(i0, P)], sc[:, hh, ds(i0, P)], tri[:],
                            op=ADD,
                        )
                ex = ex_pool.tile([P, 2, HF], FP16, tag="ex")
                nc.scalar.activation(
                    ex[:, :, ds(i0, w)], sc[:, :, ds(i0, w)], EXPF,
                    bias=loggate[:, jb : jb + 1],
                )
                if len(pend) >= 2:
                    emit_out(*pend.pop(0))
                if jb == 0:
                    nc.vector.tensor_copy(ACC[:, :, ds(ih * HF, HF)], ex[:])
                else:
                    nc.vector.tensor_tensor(
                        ACC[:, :, ds(ih * HF + i0, w)],
                        ACC[:, :, ds(ih * HF + i0, w)],
                        ex[:, :, ds(i0, w)],
                        op=ADD,
                    )
                pend.append((jb, ex, ih, i0, w, jb == jmax - 1))
                # fillers: scale count to remaining backlog
                backlog = len(wo_q) + len(proj_items)
                remaining = max(1, nslots - slot)
                nfill = 1 if backlog <= remaining else 2
                for _ in range(nfill):
                    if not filler():
                        break
                slot += 1

        while pend:
            emit_out(*pend.pop(0))
        for s in sorted(sched):  # steps not reached inside the slot loop
            for fn in sched[s]:
                fn()

        den1, norm1, wo1 = make_chain(1)
        for _ in range(2):  # filler cover for the ACC->den dependency
            if not filler():
                break
        den1()
        if nb == NB - 1:
            norm1()
            wo1()
            po_fine[0] = True
            while wo_q:
                emit_po()
        else:
            carry = [(0, norm1), (1, wo1)]
            # drain remaining projection items so chunk nb+1 is ready
            while proj_items:
                proj_items.pop(0)()
            if nb + 2 < NB:
                proj_items = make_proj_items(nb + 2)
                proj_items.pop(0)()  # chunk nb+2 input DMA a full unit early
            else:
                proj_items = []


_NC_CACHE = None


def build_nc():
    global _NC_CACHE
    if _NC_CACHE is not None:
        return _NC_CACHE
    nc = bacc.Bacc("TRN2", target_bir_lowering=False, debug=False)
    io = {
        "xt": nc.dram_tensor("xt", [D, S], BF16, kind="ExternalInput").ap(),
        "wq": nc.dram_tensor("wq", [D, 2 * HD], BF16, kind="ExternalInput").ap(),
        "wk": nc.dram_tensor("wk", [D, HD], BF16, kind="ExternalInput").ap(),
        "wv": nc.dram_tensor("wv", [D, HD], BF16, kind="ExternalInput").ap(),
        "wo": nc.dram_tensor("wo", [2 * HD, D], BF16, kind="ExternalInput").ap(),
        "tabs": nc.dram_tensor("tabs", [P, 4, S], BF16, kind="ExternalInput").ap(),
        "loggate": nc.dram_tensor("loggate", [P, NJB], F32, kind="ExternalInput").ap(),
        "tri": nc.dram_tensor("tri", [P, P], F32, kind="ExternalInput").ap(),
        "outp": nc.dram_tensor("outp", [S, D], FP16, kind="ExternalOutput").ap(),
    }
    with tile.TileContext(nc) as tc:
        _body(tc, io)
    nc.compile()
    _NC_CACHE = nc
    return nc


def make_in_maps(hidden_states, attention_mask, cos, sin, gate, Wq, Wk, Wv, Wo):
    import ml_dtypes
    bf16 = ml_dtypes.bfloat16
    X = np.asarray(hidden_states, np.float32).reshape(S, D)
    xt = np.ascontiguousarray(X.T.astype(bf16))
    cosT = np.ascontiguousarray(np.asarray(cos, np.float32).reshape(S, HD).T)
    sinT = np.ascontiguousarray(np.asarray(sin, np.float32).reshape(S, HD).T)
    sinTs = np.concatenate([-sinT[: HD // 2], sinT[HD // 2 :]], axis=0)
    sc = np.float32(1.0 / math.sqrt(HD))
    tabs = np.ascontiguousarray(
        np.stack([cosT * sc, sinTs * sc, cosT, sinTs], axis=1).astype(bf16)
    )
    # log gate with 2^-5 shift: keeps exp sums within fp16 range
    g = np.asarray(gate, np.float32).reshape(S) + np.float32(1e-8)
    lg = np.log(g).astype(np.float32) - np.float32(5.0 * math.log(2.0))
    loggate = np.ascontiguousarray(lg.reshape(NJB, P).T)
    jj = np.arange(P)[:, None]
    ii = np.arange(P)[None, :]
    tri = np.where(jj <= ii, np.float32(0), np.float32(-1e30))
    tri = np.ascontiguousarray(tri.astype(np.float32))

    Wq = np.asarray(Wq, np.float32)
    Wk = np.asarray(Wk, np.float32)
    Wv = np.asarray(Wv, np.float32)
    Wo = np.asarray(Wo, np.float32)

    in_maps = []
    for c in range(8):
        g128 = c // 2
        in_maps.append(
            {
                "xt": xt,
                "wq": np.ascontiguousarray(Wq[:, c * 256 : (c + 1) * 256].astype(bf16)),
                "wk": np.ascontiguousarray(Wk[:, g128 * HD : (g128 + 1) * HD].astype(bf16)),
                "wv": np.ascontiguousarray(Wv[:, g128 * HD : (g128 + 1) * HD].astype(bf16)),
                "wo": np.ascontiguousarray(Wo[c * 256 : (c + 1) * 256, :].astype(bf16)),
                "tabs": tabs,
                "loggate": loggate,
                "tri": tri,
            }
        )
    return in_maps


def kernel(hidden_states, attention_mask, cos, sin, gate, Wq, Wk, Wv, Wo,
           **kwargs):
    nc = build_nc()
    in_maps = make_in_maps(
        hidden_states, attention_mask, cos, sin, gate, Wq, Wk, Wv, Wo
    )
    res = run_bass_kernel_spmd(nc, in_maps, core_ids=list(range(8)), **kwargs)
    acc = res.results[0]["outp"].astype(np.float32)
    for c in range(1, 8):
        acc += res.results[c]["outp"].astype(np.float32)
    out = acc.reshape(1, S, D)
    if kwargs:
        return out, res
    return out


# revision 33
# speedup vs baseline: 1.1720x; 1.0254x over previous
"""GQA attention (16 q heads / 4 kv heads, HD=128, S=4096, D=2048) with RoPE,
causal mask, log-gate on kv positions, softmax, and output projection —
distributed over 8 NeuronCores.

Sharding: head-parallel. Core c computes q heads {2c, 2c+1} and kv head c//2.
Wq/Wk/Wv split column-wise, Wo row-wise; each core produces a partial [S, D]
output (fp16); host sums the 8 partials in fp32.

Single merged pipeline on-device (v4):
 - Attention computed transposed (scores^T [j, i], j = keys on partitions).
 - log(gate) - shift applied as the exp activation's per-partition bias, so
   denominators are plain sums of ex: accumulated on DVE in fp16 (2x mode),
   summed across partitions by M=1 matmuls, reciprocal via
   reciprocal_approx_fast, partition_broadcast on GpSimd.
 - Both heads share each scores PSUM tile ([128, 2, 256]) so one activation
   instruction computes exp for both heads of a key block (one bias column).
 - Diagonal key blocks restricted to their valid i-range; a single [128,128]
   triangular mask tile handles the diagonal itself. Upper-triangle blocks
   skipped entirely.
 - Per-half finalization: the out accumulation uses one PSUM bank per
   256-column half; each half's denominator/recip/broadcast/norm chain is
   emitted a slot after the half's last key block, so Wo output blocks become
   available mid-unit and the end-of-kernel tail only drains the final half.
 - Partial outputs stored/DMAed as fp16 (halves output HBM traffic).
 - Projections for chunk nb+1 and Wo/output evacuation are emitted as fillers
   inside the attention slot stream, keeping the PE continuously busy
   (p-state ramp to 2.4 GHz) and overlapping every engine.
"""

import math
from contextlib import ExitStack

import numpy as np

import concourse.bass as bass
import concourse.mybir as mybir
import concourse.tile as tile
from concourse import bacc, bass_isa
from concourse._compat import with_exitstack
from concourse.bass import ds
from concourse.bass_utils import run_bass_kernel_spmd
from concourse.masks import make_identity

P = 128
F = 512            # q-chunk per unit
HF = 256           # i-half per scores slot (1 PSUM bank for both heads)
S = 4096
D = 2048
HD = 128
KO = D // P        # 16 k-chunks for the projections
NB = S // F        # 8 sequence chunks
NJB = S // P       # 32 key blocks
F32 = mybir.dt.float32
BF16 = mybir.dt.bfloat16
FP16 = mybir.dt.float16
MULT = mybir.AluOpType.mult
ADD = mybir.AluOpType.add
EXPF = mybir.ActivationFunctionType.Exp


@with_exitstack
def _body(ctx: ExitStack, tc: tile.TileContext, io: dict):
    nc = tc.nc

    persist = ctx.enter_context(tc.tile_pool(name="persist", bufs=1))
    qT = persist.tile([P, 2, S], BF16, tag="qT")        # [d, h, i]
    kT = persist.tile([P, S], BF16, tag="kT")           # [d, j]
    vv = persist.tile([P, NJB, HD], FP16, tag="vv")     # [j, jb, d]
    attnT = persist.tile([P, 2, S], BF16, tag="attnT")  # [d, h, i] normalized
    loggate = persist.tile([P, NJB], F32, tag="lg")     # log(g)+shift [j, jb]
    tri = persist.tile([P, P], F32, tag="tri")          # 0 / -1e30 triangle
    ident = persist.tile([P, P], BF16, tag="ident")

    wpool = ctx.enter_context(tc.tile_pool(name="wpool", bufs=1))
    wq = wpool.tile([P, KO, 2 * HD], BF16, tag="wq")
    wq_r = io["wq"].rearrange("(ko p) m -> p ko m", p=P)
    wk = wpool.tile([P, KO, HD], BF16, tag="wk")
    wk_r = io["wk"].rearrange("(ko p) m -> p ko m", p=P)
    wv = wpool.tile([P, KO, HD], BF16, tag="wv")
    wv_r = io["wv"].rearrange("(ko p) m -> p ko m", p=P)
    wo = wpool.tile([P, 2, D], BF16, tag="wo")
    ones = persist.tile([P, 1], FP16, tag="ones")
    make_identity(nc, ident[:])
    nc.vector.memset(ones[:], 1.0)

    xt_r = io["xt"].rearrange("(ko p) s -> p ko s", p=P)  # [128, 16, 4096]

    xt_pool = ctx.enter_context(tc.tile_pool(name="xt", bufs=12))
    tab_pool = ctx.enter_context(tc.tile_pool(name="tab", bufs=4))
    rope_pool = ctx.enter_context(tc.tile_pool(name="rope", bufs=2))
    vt_pool = ctx.enter_context(tc.tile_pool(name="vt", bufs=2))
    ex_pool = ctx.enter_context(tc.tile_pool(name="ex", bufs=6))
    acc_pool = ctx.enter_context(tc.tile_pool(name="acc", bufs=2))
    r2_pool = ctx.enter_context(tc.tile_pool(name="r2", bufs=2))
    rbc_pool = ctx.enter_context(tc.tile_pool(name="rbc", bufs=2))
    ob_pool = ctx.enter_context(tc.tile_pool(name="ob", bufs=6))
    psSc = ctx.enter_context(tc.tile_pool(name="psSc", bufs=2, space="PSUM"))
    psOut = ctx.enter_context(tc.tile_pool(name="psOut", bufs=1, space="PSUM"))
    psProj = ctx.enter_context(tc.tile_pool(name="psProj", bufs=1, space="PSUM"))
    psT = ctx.enter_context(tc.tile_pool(name="psT", bufs=1, space="PSUM"))
    psWo = ctx.enter_context(tc.tile_pool(name="psWo", bufs=2, space="PSUM"))

    # ------- projection machinery: per-chunk work as a list of emission
    # closures (filler items for the attention slot stream) -------
    def make_proj_items(c, fine=False):
        """Emission closures computing qT/kT/vv for sequence chunk c."""
        sl = ds(c * F, F)
        state = {}

        def dma_x():
            sub = 1 if fine else 2
            xq = []
            for xi in range(4):
                xtile = xt_pool.tile([P, 4, F], BF16, tag="xt")
                for h in range(4 // sub):  # split across DMA queues
                    nc.sync.dma_start(
                        xtile[:, ds(h * sub, sub), :],
                        xt_r[:, ds(xi * 4 + h * sub, sub), sl],
                    )
                xq.append(xtile)
            tabs = tab_pool.tile([P, 4, F], BF16, tag="tabs")
            for h in range(4 // sub):
                nc.sync.dma_start(
                    tabs[:, ds(h * sub, sub), :], io["tabs"][:, ds(h * sub, sub), sl]
                )
            state["xq"] = xq
            state["tabs"] = tabs

        def mm_group(w_sb, m0, g, key):
            def emit():
                if g == 0:
                    state[key] = psProj.tile([P, F], F32, tag="ps", name="ps")
                ps = state[key]
                for ko in range(g * 4, g * 4 + 4):
                    nc.tensor.matmul(
                        ps[:],
                        lhsT=w_sb[:, ko, ds(m0, P)],
                        rhs=state["xq"][ko // 4][:, ko % 4, :],
                        start=(ko == 0),
                        stop=(ko == KO - 1),
                    )
            return emit

        def rope_fin(key, trow, dest):
            def emit():
                ps = state[key]
                tabs = state["tabs"]
                tmp = rope_pool.tile([P, F], BF16, tag="tmp")
                nc.scalar.copy(tmp[:], ps[:])
                rot = rope_pool.tile([P, F], BF16, tag="rot")
                nc.sync.dma_start(rot[0:64, :], tmp[64:128, :])
                nc.sync.dma_start(rot[64:128, :], tmp[0:64, :])
                t1 = rope_pool.tile([P, F], BF16, tag="t1")
                nc.vector.tensor_tensor(t1[:], tmp[:], tabs[:, trow, :], op=MULT)
                r2 = rope_pool.tile([P, F], BF16, tag="r2t")
                nc.vector.tensor_tensor(r2[:], rot[:], tabs[:, trow + 1, :], op=MULT)
                nc.vector.tensor_tensor(dest, t1[:], r2[:], op=ADD)
            return emit

        def v_fin():
            ps = state["v"]
            vT = vt_pool.tile([P, F], BF16, tag="vT")
            nc.scalar.copy(vT[:], ps[:])
            for isub in range(4):
                pt = psT.tile([P, P], BF16, tag="pt")
                nc.tensor.transpose(pt[:], vT[:, ds(isub * P, P)], ident[:])
                nc.scalar.copy(vv[:, c * 4 + isub, :], pt[:])

        items = [dma_x]
        for g in range(4):
            items.append(mm_group(wq, 0, g, "q0"))
        items.append(rope_fin("q0", 0, qT[:, 0, sl]))
        for g in range(4):
            items.append(mm_group(wq, P, g, "q1"))
        items.append(rope_fin("q1", 0, qT[:, 1, sl]))
        for g in range(4):
            items.append(mm_group(wk, 0, g, "k"))
        items.append(rope_fin("k", 2, kT[:, sl]))
        for g in range(4):
            items.append(mm_group(wv, 0, g, "v"))
        items.append(v_fin)
        return items

    # ------- Wo output machinery -------
    wo_q = []  # (nb, i2, e) output blocks, appended once their norm is emitted
    wo_alt = [0]
    po_fine = [False]  # final unit: split output DMAs finer for a short tail

    def emit_po():
        _, i2, e = wo_q.pop(0)
        po = psWo.tile([P, F], F32, tag="po")
        for hh in range(2):
            nc.tensor.matmul(
                po[:],
                lhsT=attnT[:, hh, ds(i2 * P, P)],
                rhs=wo[:, hh, ds(e * F, F)],
                start=(hh == 0),
                stop=(hh == 1),
            )
        ob = ob_pool.tile([P, F], FP16, tag="ob")
        if wo_alt[0] % 2 == 0:
            nc.scalar.copy(ob[:], po[:])
        else:
            nc.vector.tensor_copy(ob[:], po[:])
        wo_alt[0] += 1
        nsp = 4 if po_fine[0] else 2
        for h2 in range(nsp):
            nc.sync.dma_start(
                io["outp"][ds(i2 * P, P), ds(e * F + h2 * (F // nsp), F // nsp)],
                ob[:, ds(h2 * (F // nsp), F // nsp)],
            )

    # ------- prologue -------
    # DMA issue order matters: the first projection group (q0 g0/g1) needs
    # wq ko0-7 m0-255 and x chunk-0 ko0-7; put those on the first queues so
    # they land together in the first DMA round.
    proj_items = make_proj_items(0, fine=True)
    dma_x0 = proj_items.pop(0)
    for g in range(4):
        nc.sync.dma_start(wq[:, ds(g * 2, 2), :], wq_r[:, ds(g * 2, 2), :])
    for h in range(4):
        nc.sync.dma_start(wk[:, ds(h * 4, 4), :], wk_r[:, ds(h * 4, 4), :])
    dma_x0()  # 16 x sub-DMAs + 4 tabs
    for g in range(4, 8):
        nc.sync.dma_start(wq[:, ds(g * 2, 2), :], wq_r[:, ds(g * 2, 2), :])
    for h in range(4):
        nc.sync.dma_start(wv[:, ds(h * 4, 4), :], wv_r[:, ds(h * 4, 4), :])
    nc.sync.dma_start(loggate[:], io["loggate"])
    nc.sync.dma_start(tri[:], io["tri"])
    # chunk-0 emission order: q0 then k first (k's weights are small and land
    # in the first DMA round, covering the second x-DMA round's latency)
    items0 = proj_items
    reordered = items0[0:5] + items0[10:15] + items0[5:10] + items0[15:]
    for it in reordered:
        it()  # chunk 0 projections up front
    wo_rr = io["wo"].rearrange("(h p) e -> p h e", p=P)
    for g in range(4):
        nc.sync.dma_start(wo[:, :, ds(g * F, F)], wo_rr[:, :, ds(g * F, F)])
    proj_items = make_proj_items(1)
    proj_items.pop(0)()  # chunk-1 input DMA issued ahead of unit 0
    carry = []  # (delay_slots, closure) finalization steps from previous unit

    for nb in range(NB):
        out2 = [
            psOut.tile([P, 2, HF], F32, tag=f"oi{ih}", name=f"oi{ih}")
            for ih in range(2)
        ]
        ACC = acc_pool.tile([P, 2, F], FP16, tag="acc")
        pend = []  # out matmuls trail scores by 2 slots
        jmax0 = 4 * nb + 2
        nslots = jmax0 + 4 * nb + 4
        sched = {}
        for d, fn in carry:
            sched.setdefault(d, []).append(fn)
        carry = []

        def emit_out(jb, ex, ih, i0, w, last):
            for hh in range(2):
                nc.tensor.matmul(
                    out2[ih][:, hh, ds(i0, w)],
                    lhsT=vv[:, jb, :],
                    rhs=ex[:, hh, ds(i0, w)],
                    start=(jb == 0 and hh == 0),
                    stop=(last and hh == 1),
                )

        # per-half finalization chain: M=1 matmuls sum ACC over partitions
        # (both heads into one PSUM row), DVE reciprocal, one GpSimd
        # broadcast, then norm into attnT.
        def make_chain(ih, nb=nb, ACC=ACC, out2=out2):
            st = {}

            def den_step():
                den = psWo.tile([P, F], F32, tag="po", name=f"den{ih}")
                for hh in range(2):
                    nc.tensor.matmul(
                        den[0:1, ds(hh * HF, HF)],
                        lhsT=ones[:],
                        rhs=ACC[:, hh, ds(ih * HF, HF)],
                        start=(hh == 0),
                        stop=(hh == 1),
                    )
                r2 = r2_pool.tile([1, F], F32, tag="r2", name=f"r2_{ih}")
                nc.vector.reciprocal_approx_fast(r2[:], den[0:1, :])
                rbc = rbc_pool.tile([P, 2, HF], F32, tag="rbc")
                nc.gpsimd.partition_broadcast(rbc[:], r2[:])
                st["rbc"] = rbc

            def norm_step():
                nc.vector.tensor_tensor(
                    attnT[:, :, ds(nb * F + ih * HF, HF)],
                    out2[ih][:],
                    st["rbc"][:],
                    op=MULT,
                )

            def wo_step():
                base = nb * 4 + ih * 2
                wo_q.extend((nb, base + k, e) for k in range(2) for e in range(4))

            return den_step, norm_step, wo_step

        den0, norm0, wo0 = make_chain(0)
        sched.setdefault(jmax0 + 1, []).append(den0)
        sched.setdefault(jmax0 + 4, []).append(norm0)
        sched.setdefault(jmax0 + 5, []).append(wo0)

        slot = 0
        fill_alt = [0]

        def filler():
            # alternate between Wo output blocks and projection items
            a, b = (0, 1) if fill_alt[0] % 2 == 0 else (1, 0)
            for which in (a, b):
                if which == 0 and wo_q:
                    emit_po()
                    fill_alt[0] += 1
                    return True
                if which == 1 and proj_items:
                    proj_items.pop(0)()
                    fill_alt[0] += 1
                    return True
            return False

        for ih in range(2):
            jmax = 4 * nb + 2 * ih + 2
            for jb in range(jmax):
                for fn in sched.pop(slot, ()):
                    fn()
                dp = jb - (4 * nb + 2 * ih)
                i0 = max(0, dp) * P
                w = HF - i0
                qoff = nb * F + ih * HF + i0
                # scores for both heads -> one PSUM bank
                sc = psSc.tile([P, 2, HF], F32, tag="sc")
                for hh in range(2):
                    nc.tensor.matmul(
                        sc[:, hh, ds(i0, w)],
                        lhsT=kT[:, ds(jb * P, P)],
                        rhs=qT[:, hh, ds(qoff, w)],
                        start=(hh == 0),
                        stop=(hh == 1),
                    )
                if dp >= 0:  # diagonal block: triangular mask
                    for hh in range(2):
                        nc.vector.tensor_tensor(
                            sc[:, hh, ds(i0, P)], sc[:, hh, ds(i0, P)], tri[:],
                            op=ADD,
                        )
                ex = ex_pool.tile([P, 2, HF], FP16, tag="ex")
                nc.scalar.activation(
                    ex[:, :, ds(i0, w)], sc[:, :, ds(i0, w)], EXPF,
                    bias=loggate[:, jb : jb + 1],
                )
                if len(pend) >= 2:
                    emit_out(*pend.pop(0))
                if jb == 0:
                    nc.vector.tensor_copy(ACC[:, :, ds(ih * HF, HF)], ex[:])
                else:
                    nc.vector.tensor_tensor(
                        ACC[:, :, ds(ih * HF + i0, w)],
                        ACC[:, :, ds(ih * HF + i0, w)],
                        ex[:, :, ds(i0, w)],
                        op=ADD,
                    )
                pend.append((jb, ex, ih, i0, w, jb == jmax - 1))
                # fillers: scale count to remaining backlog
                backlog = len(wo_q) + len(proj_items)
                remaining = max(1, nslots - slot)
                nfill = 1 if backlog <= remaining else 2
                for _ in range(nfill):
                    if not filler():
                        break
                slot += 1

        while pend:
            emit_out(*pend.pop(0))
        for s in sorted(sched):  # steps not reached inside the slot loop
            for fn in sched[s]:
                fn()

        den1, norm1, wo1 = make_chain(1)
        for _ in range(2):  # filler cover for the ACC->den dependency
            if not filler():
                break
        den1()
        if nb == NB - 1:
            norm1()
            wo1()
            po_fine[0] = True
            while wo_q:
                emit_po()
        else:
            carry = [(0, norm1), (1, wo1)]
            # drain remaining projection items so chunk nb+1 is ready
            while proj_items:
                proj_items.pop(0)()
            if nb + 2 < NB:
                proj_items = make_proj_items(nb + 2)
                proj_items.pop(0)()  # chunk nb+2 input DMA a full unit early
            else:
                proj_items = []


_NC_CACHE = None


def build_nc():
    global _NC_CACHE
    if _NC_CACHE is not None:
        return _NC_CACHE
    nc = bacc.Bacc("TRN2", target_bir_lowering=False, debug=False)
    io = {
        "xt": nc.dram_tensor("xt", [D, S], BF16, kind="ExternalInput").ap(),
        "wq": nc.dram_tensor("wq", [D, 2 * HD], BF16, kind="ExternalInput").ap(),
        "wk": nc.dram_tensor("wk", [D, HD], BF16, kind="ExternalInput").ap(),
        "wv": nc.dram_tensor("wv", [D, HD], BF16, kind="ExternalInput").ap(),
        "wo": nc.dram_tensor("wo", [2 * HD, D], BF16, kind="ExternalInput").ap(),
        "tabs": nc.dram_tensor("tabs", [P, 4, S], BF16, kind="ExternalInput").ap(),
        "loggate": nc.dram_tensor("loggate", [P, NJB], F32, kind="ExternalInput").ap(),
        "tri": nc.dram_tensor("tri", [P, P], F32, kind="ExternalInput").ap(),
        "outp": nc.dram_tensor("outp", [S, D], FP16, kind="ExternalOutput").ap(),
    }
    with tile.TileContext(nc) as tc:
        _body(tc, io)
    nc.compile()
    _NC_CACHE = nc
    return nc


def make_in_maps(hidden_states, attention_mask, cos, sin, gate, Wq, Wk, Wv, Wo):
    import ml_dtypes
    bf16 = ml_dtypes.bfloat16
    X = np.asarray(hidden_states, np.float32).reshape(S, D)
    xt = np.ascontiguousarray(X.T.astype(bf16))
    cosT = np.ascontiguousarray(np.asarray(cos, np.float32).reshape(S, HD).T)
    sinT = np.ascontiguousarray(np.asarray(sin, np.float32).reshape(S, HD).T)
    sinTs = np.concatenate([-sinT[: HD // 2], sinT[HD // 2 :]], axis=0)
    sc = np.float32(1.0 / math.sqrt(HD))
    tabs = np.ascontiguousarray(
        np.stack([cosT * sc, sinTs * sc, cosT, sinTs], axis=1).astype(bf16)
    )
    # log gate with 2^-5 shift: keeps exp sums within fp16 range
    g = np.asarray(gate, np.float32).reshape(S) + np.float32(1e-8)
    lg = np.log(g).astype(np.float32) - np.float32(5.0 * math.log(2.0))
    loggate = np.ascontiguousarray(lg.reshape(NJB, P).T)
    jj = np.arange(P)[:, None]
    ii = np.arange(P)[None, :]
    tri = np.where(jj <= ii, np.float32(0), np.float32(-1e30))
    tri = np.ascontiguousarray(tri.astype(np.float32))

    Wq = np.asarray(Wq, np.float32)
    Wk = np.asarray(Wk, np.float32)
    Wv = np.asarray(Wv, np.float32)
    Wo = np.asarray(Wo, np.float32)

    in_maps = []
    for c in range(8):
        g128 = c // 2
        in_maps.append(
            {
                "xt": xt,
                "wq": np.ascontiguousarray(Wq[:, c * 256 : (c + 1) * 256].astype(bf16)),
                "wk": np.ascontiguousarray(Wk[:, g128 * HD : (g128 + 1) * HD].astype(bf16)),
                "wv": np.ascontiguousarray(Wv[:, g128 * HD : (g128 + 1) * HD].astype(bf16)),
                "wo": np.ascontiguousarray(Wo[c * 256 : (c + 1) * 256, :].astype(bf16)),
                "tabs": tabs,
                "loggate": loggate,
                "tri": tri,
            }
        )
    return in_maps


def kernel(hidden_states, attention_mask, cos, sin, gate, Wq, Wk, Wv, Wo,
           **kwargs):
    nc = build_nc()
    in_maps = make_in_maps(
        hidden_states, attention_mask, cos, sin, gate, Wq, Wk, Wv, Wo
    )
    res = run_bass_kernel_spmd(nc, in_maps, core_ids=list(range(8)), **kwargs)
    acc = res.results[0]["outp"].astype(np.float32)
    for c in range(1, 8):
        acc += res.results[c]["outp"].astype(np.float32)
    out = acc.reshape(1, S, D)
    if kwargs:
        return out, res
    return out
